# revision 21
# baseline (speedup 1.0000x reference)
"""Trainium2 Bass kernel for segment min/max/mean stats + bounds gather.

Strategy (label-space sharding; host routes, device reduces):
  * Host routes every element twice (by cell_1 label and by cell_2 label)
    into 8 per-core label ranges; each core computes exact stats for its
    ranges - no cross-core reduction.
  * Elements are quantized to int16 (x_q = rint(x * S), S = 32000/max|x|).
    Min/max/sum in the scaled integer domain are exact in fp32 (sums stay
    below 2^24); host divides by S afterwards. Halves DMA vs fp32.
  * Within a core, each label's elements pack into fixed-width rows
    (W in multiples of 6; the last 2 pairs of each row are reserved pad,
    so usable capacity is W-4; pads repeat the last element and the sum
    is corrected on the host).
  * Device: ONE hand-built 5-uop custom DVE op per tile computes min,
    max and sum scans simultaneously in a single 2-stream pass (both DVE
    read ports busy, 1 pair/cycle/lane), writing a rotating
    min,max,sum,min,... stream per page. The last 3 entries of each page
    hold the complete stats; a tiny strided copy extracts them before
    DMA-out.
"""

import os
from dataclasses import dataclass

import numpy as np

N_CORES = 8
C = 8
C1 = 400_000
C2 = 100_000
# W multiples of 6 (3-phase rotation needs W/2 % 3 == 0); capacity = W.
# The device min misses the last 2 pairs and max the last pair of each row;
# the host patches both from the known elements at those slots.
CLASSES_C1 = (6, 12, 18, 24, 30, 36, 42, 48)
CLASSES_C2 = (54, 60, 66, 72, 78, 84, 90, 96, 102, 108, 114, 120, 126)
TILE_IN_BYTES = int(os.environ.get("KERNEL_TILE_BYTES", 3 << 20))

_compiled_cache = {}
_rot_op = None
last_exec_time_ns = None
last_trace_path = None


# --------------------------------------------------------------------------
# Hand-built 5-uop custom DVE op: min/max/sum scans in one 2-stream pass.
# uops: 0=first-elem(step-like, runs once), 1=A(write min), 2=B(write max),
#       3=C(write sum), 4=step(page boundary; drops scan history).
# Page of N pairs emits a rotating stream min,max,sum,... ; with N%3==0 the
# final 3 writes are min(pairs 0..N-3), max(0..N-2), sum(0..N-1).
# --------------------------------------------------------------------------
def _get_rot_op():
    global _rot_op
    if _rot_op is not None:
        return _rot_op
    import concourse.dve_ops as DO
    from concourse.dve_ops import OPS, DveOp, CUSTOM_DVE_SPECS
    from concourse.dve_spec import (
        AluOp, Spec, Src0, Src1, minn, scan, C0,
        _Stage, _State, _Placement, _assemble, PREV,
    )
    from concourse.dve_uop import (
        DELAY_OUT, OutSel, AluInp, Trigger, DveOpSpec,
    )

    name = "ROWMMS3_ANT"
    if name in DO._SUB_OPCODE_FOR_NAME:
        _rot_op = next(op for op in OPS if op.name == name)
        return _rot_op

    L_S0, L_S1, L_SM, L_SX = 0, 1, 2, 3
    # sentinel nodes for captured scan values: non-Leaf lane keys are skipped
    # by the inp-crossbar wiring but enable lane pass-through in _emit_dp
    sm_node = scan(AluOp.MIN, minn(Src0, Src1), init=C0)
    sx_node = scan(AluOp.MAX, minn(Src0, Src1), init=C0)
    lane = {Src0: L_S0, Src1: L_S1, sm_node: L_SM, sx_node: L_SX}
    S0i, S1i = AluInp.PREV_DELAY_0, AluInp.PREV_DELAY_1
    pipeline = [
        _Stage(AluOp.MIN, S0i, S1i),
        _Stage(AluOp.MIN, AluInp.CURR_ALU_OUT, PREV),
        _Stage(AluOp.MAX, S0i, S1i),
        _Stage(AluOp.MAX, AluInp.CURR_ALU_OUT, PREV),
        _Stage(AluOp.ADD, S0i, S1i),
        _Stage(AluOp.ADD, AluInp.CURR_ALU_OUT, PREV),
        _Stage(AluOp.BYPASS, PREV),
        _Stage(AluOp.BYPASS, PREV),
    ]
    captures = [(2, L_SM), (4, L_SX)]

    def mk_placement(out_sel):
        return _Placement(
            pipeline=pipeline, node_stage={}, lane=lane,
            out_sel=out_sel, accum_stage=None, captures=captures,
        )

    p_min = mk_placement(DELAY_OUT[L_SM])
    p_max = mk_placement(DELAY_OUT[L_SX])
    p_sum = mk_placement(OutSel.ALU_OUT)
    step_ov = {
        1: _Stage(AluOp.BYPASS, PREV),
        3: _Stage(AluOp.BYPASS, PREV),
        5: _Stage(AluOp.BYPASS, PREV),
    }
    TRIG = (Trigger.SRC_TENSOR_DONE, Trigger.SUB_DIM_DONE, Trigger.COUNT)
    consume = (True, True)
    states = [
        _State(placement=p_min, trigger=TRIG, next=(0, 4, 2), repeat=1,
               consume=consume, overrides=step_ov),
        _State(placement=p_min, trigger=TRIG, next=(0, 4, 2), repeat=1,
               consume=consume),
        _State(placement=p_max, trigger=TRIG, next=(0, 4, 3), repeat=1,
               consume=consume),
        _State(placement=p_sum, trigger=TRIG, next=(0, 4, 1), repeat=1,
               consume=consume),
        _State(placement=p_min, trigger=TRIG, next=(0, 4, 2), repeat=1,
               consume=consume, overrides=step_ov),
    ]
    uops = [_assemble(s) for s in states]

    @dataclass(frozen=True)
    class DveOpManual(DveOp):
        def compile(self, ver):
            from concourse.dve_ops import _COMPILE_CACHE
            key = (self.name, ver)
            if (r := _COMPILE_CACHE.get(key)) is not None:
                return r
            result = DveOpSpec(
                name=self.name,
                opcode=DO.get_dve_sub_opcode(self.name),
                uops=uops,
                rd1_en=True,
            )
            _COMPILE_CACHE[key] = result
            return result

    spec = Spec(
        body=scan(AluOp.MIN, minn(Src0, Src1), init=C0),
        reference=lambda in0, in1, c0, c1, c2: np.minimum.accumulate(
            np.minimum(in0, in1), axis=-1),
    )
    op = DveOpManual(name, spec, subdim=True, uops_sha={})
    OPS.append(op)
    CUSTOM_DVE_SPECS[name] = spec
    DO._SUB_OPCODE_FOR_NAME[name] = DO._CUSTOM_DVE_ROW_BASE + len(OPS) - 1
    assert DO._SUB_OPCODE_FOR_NAME[name] < 0x20, "custom DVE row overflow"
    _rot_op = op
    return op


# --------------------------------------------------------------------------
# Host-side layout
# --------------------------------------------------------------------------
def _build_layout(counts, starts, order, num_labels, classes):
    """Pack labels into fixed-width slot rows. Returns per-class dicts."""
    caps = np.array(classes)
    cap_max = caps[-1]
    n_full = np.maximum(0, counts - 1) // cap_max
    out = []
    for ci, W in enumerate(classes):
        rem = counts - n_full * cap_max
        cls_idx = np.searchsorted(caps, rem)
        sel = np.nonzero((cls_idx == ci) & (counts > 0))[0]
        r_off = starts[sel] + n_full[sel] * cap_max
        r_cnt = counts[sel] - n_full[sel] * cap_max
        col = np.arange(W)[None, :]
        idx_in_order = r_off[:, None] + np.minimum(col, (r_cnt - 1)[:, None])
        rows_idx = order[idx_in_order]
        rows_padcnt = (W - r_cnt).astype(np.int64)
        rows_label = sel
        if ci == len(classes) - 1:
            split_lab = np.nonzero(n_full > 0)[0]
            if len(split_lab):
                nf = n_full[split_lab]
                tot = int(nf.sum())
                row_lab = np.repeat(split_lab, nf)
                row_ord = np.arange(tot) - np.repeat(
                    np.concatenate([[0], np.cumsum(nf)[:-1]]), nf
                )
                f_off = starts[row_lab] + row_ord * cap_max
                fidx = order[f_off[:, None] + np.arange(W)[None, :]]
                rows_idx = np.concatenate([rows_idx, fidx], axis=0)
                rows_padcnt = np.concatenate(
                    [rows_padcnt, np.zeros(tot, dtype=np.int64)]
                )
                rows_label = np.concatenate([rows_label, row_lab])
        rows_core = rows_label * N_CORES // num_labels
        o = np.argsort(rows_core, kind="stable")
        out.append(
            dict(
                W=W,
                rows_label=rows_label[o],
                rows_idx=rows_idx[o],
                rows_padcnt=rows_padcnt[o],
                per_core=np.bincount(rows_core[o], minlength=N_CORES),
            )
        )
    return out


def _tile_plan(W, max_rows):
    """List of per-tile R values covering >= max_rows, 128-row granular."""
    r_big = max(1, TILE_IN_BYTES // (128 * C * W * 2))
    lines = -(-max_rows // 128)
    rs = []
    while lines > 0:
        r = min(r_big, lines)
        rs.append(r)
        lines -= r
    return rs


# --------------------------------------------------------------------------
# Device program
# --------------------------------------------------------------------------
def _build_program(block_shapes, copy_engine):
    import concourse.bacc as bacc
    import concourse.mybir as mybir
    import concourse.tile as tile

    op = _get_rot_op()

    nc = bacc.Bacc("TRN2", target_bir_lowering=False, debug=False, num_devices=N_CORES)
    tensors = []
    for name, cap, W, rs in block_shapes:
        din = nc.dram_tensor(f"in_{name}", [cap, C, W], mybir.dt.int16, kind="ExternalInput")
        dout = nc.dram_tensor(f"o_{name}", [cap, C, 3], mybir.dt.float32, kind="ExternalOutput")
        tensors.append((din, dout))

    with tile.TileContext(nc) as tc:
        io_bufs = int(os.environ.get("KERNEL_IO_BUFS", 4))
        dump_bufs = int(os.environ.get("KERNEL_DUMP_BUFS", 3))
        with (
            tc.tile_pool(name="io", bufs=io_bufs) as pool,
            tc.tile_pool(name="dump", bufs=dump_bufs) as dpool,
            tc.tile_pool(name="out", bufs=4) as opool,
        ):
            for (name, cap, W, rs), (din, dout) in zip(block_shapes, tensors):
                N = W // 2
                row0 = 0
                for R in rs:
                    nrows = 128 * R
                    din_t = din.ap()[row0 : row0 + nrows].rearrange(
                        "(p r) c w -> p r c w", p=128, r=R
                    )
                    dout_t = dout.ap()[row0 : row0 + nrows].rearrange(
                        "(p r) c v -> p r c v", p=128, r=R
                    )
                    row0 += nrows
                    tl = pool.tile([128, R, C, W], mybir.dt.int16, tag="in")
                    nc.sync.dma_start(tl[:], din_t)
                    dump = dpool.tile([128, R, C, N], mybir.dt.float32, tag="dump")
                    ot = opool.tile([128, R, C, 3], mybir.dt.float32, tag="out")
                    tv = tl[:].rearrange("p r c w -> p (r c) w")
                    dv = dump[:].rearrange("p r c n -> p (r c) n")
                    nc.vector._custom_dve(op, out=dv, in0=tv[:, :, 0:N], in1=tv[:, :, N:W])
                    ov = ot[:].rearrange("p r c v -> p (r c) v")
                    if copy_engine == "scalar":
                        nc.scalar.copy(ov, dv[:, :, N - 3 : N])
                    else:
                        nc.vector.tensor_copy(ov, dv[:, :, N - 3 : N])
                    nc.sync.dma_start(dout_t, ot[:])
    nc.compile()
    return nc


# --------------------------------------------------------------------------
# Marshalling + epilogue
# --------------------------------------------------------------------------
def _pack_core_inputs(xq, lay, caps):
    per_core = [dict() for _ in range(N_CORES)]
    for blk, cap in zip(lay, caps):
        W = blk["W"]
        pc = blk["per_core"]
        offs = np.concatenate([[0], np.cumsum(pc)])
        for k in range(N_CORES):
            n = int(pc[k])
            buf = np.zeros((cap, C, W), dtype=np.int16)
            if n:
                idx = blk["rows_idx"][offs[k] : offs[k] + n]
                buf[:n] = xq[idx].transpose(0, 2, 1)
            per_core[k][f"W{W}"] = buf
    return per_core


def _combine(xq, lay, results, num_labels, sizes, scale):
    mn = np.full((num_labels, C), np.inf, np.float32)
    mx = np.full((num_labels, C), -np.inf, np.float32)
    sm = np.zeros((num_labels, C), np.float64)
    for blk in lay:
        W = blk["W"]
        N = W // 2
        pc = blk["per_core"]
        r = np.concatenate(
            [results[k][f"W{W}"][: pc[k]] for k in range(N_CORES)], axis=0
        )
        lab = blk["rows_label"]
        pad = blk["rows_padcnt"].astype(np.float64)
        padval = xq[blk["rows_idx"][:, -1]].astype(np.float64)
        r_sm = r[:, :, 2].astype(np.float64) - pad[:, None] * padval
        # device min misses pairs N-2,N-1 (slots N-2,N-1,W-2,W-1); max misses
        # pair N-1 (slots N-1,W-1) - patch from the known slot elements
        tail4 = xq[blk["rows_idx"][:, [N - 2, N - 1, W - 2, W - 1]]].astype(
            np.float32
        )
        r_mn = np.minimum(r[:, :, 0], tail4.min(axis=1))
        r_mx = np.maximum(r[:, :, 1], tail4[:, (1, 3)].max(axis=1))
        np.minimum.at(mn, lab, r_mn)
        np.maximum.at(mx, lab, r_mx)
        np.add.at(sm, lab, r_sm)
    szf = sizes.astype(np.float64)
    inv = 1.0 / scale
    with np.errstate(divide="ignore", invalid="ignore"):
        mean = (sm * inv / szf[:, None]).astype(np.float32)
    s = np.exp(-sizes.astype(np.float32)) - 0.5
    return np.concatenate(
        [mn * np.float32(inv), mx * np.float32(inv), mean, s[:, None]], axis=1
    )


def kernel(input, cell_1_mask, cell_2_mask, cell_1_bounds, cell_1_sizes,
           cell_2_sizes, **_ignored):
    global last_exec_time_ns, last_trace_path

    from concourse.bass_utils import run_bass_kernel_spmd

    x = np.asarray(input, dtype=np.float32)
    amax = float(np.abs(x).max())
    scale = 32000.0 / amax if amax > 0 else 1.0
    xq = np.rint(x * scale).astype(np.int16)

    layouts = []
    for mask, num, classes in (
        (cell_1_mask, C1, CLASSES_C1),
        (cell_2_mask, C2, CLASSES_C2),
    ):
        l = np.asarray(mask).astype(np.int64) - 1
        valid = (l >= 0) & (l < num)
        if not valid.all():
            lv = l[valid]
            pos = np.nonzero(valid)[0]
        else:
            lv, pos = l, None
        counts = np.bincount(lv, minlength=num)
        order = np.argsort(lv, kind="stable")
        if pos is not None:
            order = pos[order]
        starts = np.concatenate([[0], np.cumsum(counts)[:-1]])
        layouts.append(_build_layout(counts, starts, order, num, classes))
    lay1, lay2 = layouts

    block_shapes = []
    caps1, caps2 = [], []
    for tag, lay, caps in (("c1", lay1, caps1), ("c2", lay2, caps2)):
        for blk in lay:
            W = blk["W"]
            maxrows = int(np.max(blk["per_core"]))
            rs = tuple(_tile_plan(W, maxrows))
            cap = 128 * sum(rs)
            caps.append(cap)
            block_shapes.append((f"{tag}W{W}", cap, W, rs))
    # biggest blocks first: the kernel tail (compute after the last DMA-in)
    # then falls on the smallest classes
    block_shapes.sort(key=lambda b: -b[1] * b[2])
    # split the very first tile so compute starts before a full-size DMA lands
    name0, cap0, w0, rs0 = block_shapes[0]
    if rs0 and rs0[0] >= 4:
        q = rs0[0] // 4
        block_shapes[0] = (name0, cap0, w0, (q, rs0[0] - q) + rs0[1:])

    copy_engine = os.environ.get("KERNEL_COPY_ENGINE", "scalar")
    key = (copy_engine, tuple(block_shapes))
    if key not in _compiled_cache:
        _compiled_cache[key] = _build_program(block_shapes, copy_engine)
    nc = _compiled_cache[key]

    core_in1 = _pack_core_inputs(xq, lay1, caps1)
    core_in2 = _pack_core_inputs(xq, lay2, caps2)
    in_maps = []
    for k in range(N_CORES):
        m = {}
        for blk in lay1:
            m[f"in_c1W{blk['W']}"] = core_in1[k][f"W{blk['W']}"]
        for blk in lay2:
            m[f"in_c2W{blk['W']}"] = core_in2[k][f"W{blk['W']}"]
        in_maps.append(m)

    trace = bool(int(os.environ.get("KERNEL_TRACE", "0")))
    if trace:
        try:
            import ntff_shim

            ntff_shim.install()
        except Exception:
            trace = False
    res = None
    for attempt in range(4):
        try:
            res = run_bass_kernel_spmd(
                nc, in_maps, core_ids=list(range(N_CORES)), trace=trace and attempt < 2
            )
            break
        except Exception:
            if attempt == 3:
                raise
            import time as _time

            _time.sleep(15)
    last_exec_time_ns = res.exec_time_ns
    last_trace_path = (
        res.instructions_and_trace[1] if res.instructions_and_trace else None
    )

    def rename(lay, tag):
        return [
            {f"W{blk['W']}": res.results[k][f"o_{tag}W{blk['W']}"] for blk in lay}
            for k in range(N_CORES)
        ]

    c1_stats = _combine(xq, lay1, rename(lay1, "c1"), C1,
                        np.asarray(cell_1_sizes), scale)
    c2_stats = _combine(xq, lay2, rename(lay2, "c2"), C2,
                        np.asarray(cell_2_sizes), scale)

    b = np.asarray(cell_1_bounds).astype(np.int64)
    u = np.clip(b[:, 0] - 1, -C2, C2 - 1)
    v = np.clip(b[:, 1] - 1, -C2, C2 - 1)
    return c1_stats, c2_stats[u], c2_stats[v]
